# revision 12
# baseline (speedup 1.0000x reference)
"""Trainium2 Bass kernel for nn_Attention_2052994367753.

Math (per batch b):
    q = x @ Wq.T + bq ; k = x @ Wk.T + bk ; v = x @ Wv.T + bv          [32, 2048]
    scores[i,j] = q[i]*k[j]  (rank-1); softmax over j; sa[j] = sum_i v[i]*attn[i,j]
    out = sa + x

Key reductions used here:
  - |q*k| <= ~10 so softmax max-subtraction is unnecessary in fp32:
        E = exp(outer(q, k)); Z = rowsum(E); sa = E.T @ (v / Z)
  - E is produced directly by the ScalarE activation (exp with per-partition
    scale = q_i, input = broadcast k row) with the fused accum_out giving Z.
  - E is written in fp16 (PE runs fp16 at 1 cyc/row; absmax out err ~1.6e-4).
    v is pre-scaled by 256 on the host so w = v*256/Z stays in fp16 normal
    range; the 1/256 is folded into the PSUM->SBUF epilogue.

Sharding over 8 cores:
  - Projections: weight-sharded. Core c loads rows [256c, 256c+256) of
    Wq/Wk/Wv (as a [2048, 768] rhs block) and computes q/k/v[:, slice] for
    all 32 batches. One AllToAll then redistributes so core c holds full
    q/k/v for batches [4c, 4c+4). Attention is batch-parallel (4 per core).
"""
import numpy as np
from contextlib import ExitStack

import concourse.bass as bass
from concourse import bacc, mybir
import concourse.tile as tile
from concourse.bass_utils import run_bass_kernel_spmd

F = mybir.ActivationFunctionType
DT = mybir.dt
OP = mybir.AluOpType

SEQ = 2048
B = 32
NCORES = 8
SL = SEQ // NCORES          # 256: per-core projection output slice
BL = B // NCORES            # 4: batches per core for attention
KCH = SEQ // 128            # 16 contraction chunks
VS = 256.0                  # v prescale to keep fp16 w in normal range

_CACHE = {}


def _build():
    nc = bacc.Bacc("TRN2", target_bir_lowering=False, debug=False,
                   num_devices=NCORES)
    # bf16-split operands: cols 0:B/0:768 are hi, B:2B/768:1536 are lo.
    # x = hi + lo and W = hi + lo exactly to ~2^-17; the three bf16 matmuls
    # hi@hi + hi@lo + lo@hi reproduce the fp32 product to ~1e-7 while the PE
    # streams bf16 at 1 cyc/row (fp32 is 4).
    xT_d = nc.dram_tensor("xT", [SEQ, 2 * B], DT.bfloat16, kind="ExternalInput")
    w3_d = nc.dram_tensor("w3", [SEQ, 2 * 3 * SL], DT.bfloat16,
                          kind="ExternalInput")
    b3_d = nc.dram_tensor("b3", [1, 3 * SL], DT.float32, kind="ExternalInput")
    xloc_d = nc.dram_tensor("xloc", [BL, SEQ], DT.float32, kind="ExternalInput")
    out_d = nc.dram_tensor("out", [BL, SEQ], DT.float32, kind="ExternalOutput")

    cc_in = nc.dram_tensor("cc_in", [B, 3 * SL], DT.float32)
    cc_out = nc.dram_tensor("cc_out", [B, 3 * SL], DT.float32)

    with tile.TileContext(nc) as tc, ExitStack() as ctx:
        const_pool = ctx.enter_context(tc.tile_pool(name="const", bufs=1))

        # ---------------- phase 1: projections (weight-sharded) -------------
        xt = const_pool.tile([128, KCH * 2 * B], DT.bfloat16)
        nc.sync.dma_start(
            xt[:].rearrange("p (kc m) -> p kc m", kc=KCH),
            xT_d.ap().rearrange("(kc p) m -> p kc m", p=128),
        )
        b3t = const_pool.tile([B, 3 * SL], DT.float32)
        nc.sync.dma_start(b3t[:], b3_d.ap().partition_broadcast(B))

        qkv = const_pool.tile([B, 3 * SL], DT.float32)
        KG = 4  # k-chunks per DMA group
        with tc.tile_pool(name="w3p", bufs=2) as wpool, \
             tc.tile_pool(name="psp", bufs=1, space="PSUM") as pp:
            ps = [pp.tile([B, 384], DT.float32, tag=f"ps{h}", name=f"ps{h}")
                  for h in range(2)]
            for g in range(KCH // KG):
                w3g = wpool.tile([128, KG * 1536], DT.bfloat16)
                nc.sync.dma_start(
                    w3g[:].rearrange("p (kc n) -> p kc n", kc=KG),
                    w3_d.ap()[g * KG * 128:(g + 1) * KG * 128, :]
                        .rearrange("(kc p) n -> p kc n", p=128))
                for kci in range(KG):
                    kc = g * KG + kci
                    xh = xt[:, kc * 2 * B:kc * 2 * B + B]
                    xl = xt[:, kc * 2 * B + B:(kc + 1) * 2 * B]
                    for h in range(2):
                        wh = w3g[:, kci * 1536 + h * 384:kci * 1536 + (h + 1) * 384]
                        wl = w3g[:, kci * 1536 + 768 + h * 384:
                                 kci * 1536 + 768 + (h + 1) * 384]
                        for t, (lhsT, rhs) in enumerate(
                                [(xh, wh), (xh, wl), (xl, wh)]):
                            nc.tensor.matmul(
                                ps[h][:], lhsT, rhs,
                                start=(kc == 0 and t == 0),
                                stop=(kc == KCH - 1 and t == 2),
                            )
            for h in range(2):
                nc.vector.tensor_add(
                    qkv[:, h * 384:(h + 1) * 384], ps[h][:],
                    b3t[:, h * 384:(h + 1) * 384])
        nc.sync.dma_start(cc_in.ap(), qkv[:])

        # ---------------- AllToAll: redistribute to batch-parallel ----------
        nc.gpsimd.collective_compute(
            "AllToAll",
            OP.bypass,
            replica_groups=[list(range(NCORES))],
            ins=[cc_in.ap()],
            outs=[cc_out.ap()],
        )
        # core c now holds, for each d: cc_out[4d + i] =
        #   (q | k | v')[batch 4c + i, 256d:256d+256]
        cc = cc_out.ap()

        # ---------------- phase 2: attention (batch-parallel) ---------------
        apool = ctx.enter_context(tc.tile_pool(name="attn", bufs=2))
        epool = ctx.enter_context(tc.tile_pool(name="etile", bufs=3))
        with tc.tile_pool(name="psa", bufs=2, space="PSUM") as spp:
            for i in range(BL):
                # q in cols 0:16, v' in 16:32 ; i-index mapping: i = p*16 + c
                qv = apool.tile([128, 32], DT.float32, tag="qv")
                kb = apool.tile([128, SEQ], DT.float32, tag="kb")
                nc.sync.dma_start(
                    qv[:, 0:16],
                    cc[:, 0:SL].rearrange(
                        "(d i) (pp c) -> i d pp c", i=BL, c=16)[i:i + 1])
                nc.sync.dma_start(
                    qv[:, 16:32],
                    cc[:, 2 * SL:3 * SL].rearrange(
                        "(d i) (pp c) -> i d pp c", i=BL, c=16)[i:i + 1])
                # k broadcast to all 128 partitions; free dim j = 256d + o
                nc.sync.dma_start(
                    kb[:],
                    cc[:, SL:2 * SL].rearrange(
                        "(d i) o -> i d o", i=BL)[i:i + 1].partition_broadcast(128))

                zt = apool.tile([128, KCH], DT.float32, tag="zt")
                rz = apool.tile([128, KCH], DT.float32, tag="rz")
                wt = apool.tile([128, KCH], DT.float16, tag="wt")
                psum = spp.tile([1, SEQ], DT.float32)
                for c in range(KCH):
                    e16 = epool.tile([128, SEQ], DT.float16)
                    nc.scalar.activation(
                        e16[:], kb[:], F.Exp,
                        scale=qv[:, c:c + 1], accum_out=zt[:, c:c + 1])
                    nc.vector.reciprocal(rz[:, c:c + 1], zt[:, c:c + 1])
                    nc.vector.tensor_mul(
                        wt[:, c:c + 1], qv[:, 16 + c:17 + c], rz[:, c:c + 1])
                    for js in range(4):
                        nc.tensor.matmul(
                            psum[0:1, js * 512:(js + 1) * 512],
                            wt[:, c:c + 1],
                            e16[:, js * 512:(js + 1) * 512],
                            start=(c == 0), stop=(c == KCH - 1),
                        )
                # epilogue: out_row = psum/VS + x_row  (all at partition 0;
                # DMA handles row placement in DRAM)
                xrow = apool.tile([1, SEQ], DT.float32, tag="xrow")
                nc.sync.dma_start(xrow[:], xloc_d.ap()[i:i + 1, :])
                sarow = apool.tile([1, SEQ], DT.float32, tag="sarow")
                nc.vector.tensor_scalar(
                    sarow[:], psum[0:1, :], 1.0 / VS, None, op0=OP.mult)
                orow = apool.tile([1, SEQ], DT.float32, tag="orow")
                nc.vector.tensor_add(orow[:], sarow[:], xrow[:])
                nc.sync.dma_start(out_d.ap()[i:i + 1, :], orow[:])
    nc.compile()
    return nc


def _hilo(a):
    """Split fp32 array into (hi, lo) bf16 with a == hi + lo to ~2^-17."""
    import ml_dtypes
    hi = a.astype(ml_dtypes.bfloat16)
    lo = (a - hi.astype(np.float32)).astype(ml_dtypes.bfloat16)
    return hi, lo


def _prep_inputs(x, Wq, bq, Wk, bk, Wv, bv):
    x = np.ascontiguousarray(x, dtype=np.float32)
    xh, xl = _hilo(x.T)
    xT = np.concatenate([xh, xl], axis=1)  # [2048, 64] bf16 (hi | lo)
    in_maps = []
    for c in range(NCORES):
        sl = slice(SL * c, SL * (c + 1))
        w3 = np.concatenate(
            [Wq[sl].T, Wk[sl].T, (Wv[sl] * VS).T], axis=1)  # [2048, 768] f32
        wh, wl = _hilo(np.ascontiguousarray(w3, dtype=np.float32))
        w3b = np.concatenate([wh, wl], axis=1)  # [2048, 1536] bf16 (hi | lo)
        b3 = np.concatenate([bq[sl], bk[sl], bv[sl] * VS])[None, :]
        in_maps.append({
            "xT": np.ascontiguousarray(xT),
            "w3": np.ascontiguousarray(w3b),
            "b3": np.ascontiguousarray(b3, dtype=np.float32),
            "xloc": np.ascontiguousarray(x[BL * c:BL * (c + 1)]),
        })
    return in_maps


def run_on_device(x, Wq, bq, Wk, bk, Wv, bv, **spmd_kwargs):
    if "nc" not in _CACHE:
        _CACHE["nc"] = _build()
    nc = _CACHE["nc"]
    in_maps = _prep_inputs(x, Wq, bq, Wk, bk, Wv, bv)
    res = run_bass_kernel_spmd(nc, in_maps, core_ids=list(range(NCORES)),
                               **spmd_kwargs)
    out = np.concatenate([res.results[c]["out"] for c in range(NCORES)], axis=0)
    return np.ascontiguousarray(out, dtype=np.float32), res


def kernel(x, Wq, bq, Wk, bk, Wv, bv):
    out, _ = run_on_device(x, Wq, bq, Wk, bk, Wv, bv)
    return out


# revision 13
# speedup vs baseline: 1.0517x; 1.0517x over previous
"""Trainium2 Bass kernel for nn_Attention_2052994367753.

Math (per batch b):
    q = x @ Wq.T + bq ; k = x @ Wk.T + bk ; v = x @ Wv.T + bv          [32, 2048]
    scores[i,j] = q[i]*k[j]  (rank-1); softmax over j; sa[j] = sum_i v[i]*attn[i,j]
    out = sa + x

Key reductions used here:
  - |q*k| <= ~10 so softmax max-subtraction is unnecessary in fp32:
        E = exp(outer(q, k)); Z = rowsum(E); sa = E.T @ (v / Z)
  - E is produced directly by the ScalarE activation (exp with per-partition
    scale = q_i, input = broadcast k row) with the fused accum_out giving Z.
  - E is written in fp16 (PE runs fp16 at 1 cyc/row; absmax out err ~1.6e-4).
    v is pre-scaled by 256 on the host so w = v*256/Z stays in fp16 normal
    range; the 1/256 is folded into the PSUM->SBUF epilogue.
  - Projections run as bf16 hi/lo splits (x=xh+xl, W=Wh+Wl; hi@hi+hi@lo+lo@hi)
    which matches fp32 to ~1e-7 while streaming the PE at bf16 rate.

Sharding over 8 cores:
  - Projections are weight-sharded: core c computes q/k/v[:, 256c:256c+256]
    for all 32 batches. Two AllToAlls redistribute to batch-parallel form
    (4 batches per core): the q/k AllToAll launches first and gates the
    attention phase; the v AllToAll overlaps with early attention.
"""
import numpy as np
from contextlib import ExitStack

import concourse.bass as bass
from concourse import bacc, mybir
import concourse.tile as tile
from concourse.bass_utils import run_bass_kernel_spmd

F = mybir.ActivationFunctionType
DT = mybir.dt
OP = mybir.AluOpType

SEQ = 2048
B = 32
NCORES = 8
SL = SEQ // NCORES          # 256: per-core projection output slice
BL = B // NCORES            # 4: batches per core for attention
KCH = SEQ // 128            # 16 contraction chunks
VS = 256.0                  # v prescale to keep fp16 w in normal range

_CACHE = {}


def _build():
    nc = bacc.Bacc("TRN2", target_bir_lowering=False, debug=False,
                   num_devices=NCORES)
    # bf16 hi/lo packed operands (cols: hi block then lo block)
    xT_d = nc.dram_tensor("xT", [SEQ, 2 * B], DT.bfloat16, kind="ExternalInput")
    wqk_d = nc.dram_tensor("wqk", [SEQ, 1024], DT.bfloat16, kind="ExternalInput")
    wv_d = nc.dram_tensor("wv", [SEQ, 512], DT.bfloat16, kind="ExternalInput")
    b3_d = nc.dram_tensor("b3", [1, 3 * SL], DT.float32, kind="ExternalInput")
    xloc_d = nc.dram_tensor("xloc", [BL, SEQ], DT.float32, kind="ExternalInput")
    out_d = nc.dram_tensor("out", [BL, SEQ], DT.float32, kind="ExternalOutput")

    cc1_in = nc.dram_tensor("cc1_in", [B, 2 * SL], DT.float32)
    cc1_out = nc.dram_tensor("cc1_out", [B, 2 * SL], DT.float32)
    cc2_in = nc.dram_tensor("cc2_in", [B, SL], DT.float32)
    cc2_out = nc.dram_tensor("cc2_out", [B, SL], DT.float32)

    with tile.TileContext(nc) as tc, ExitStack() as ctx:
        const_pool = ctx.enter_context(tc.tile_pool(name="const", bufs=1))

        # ---------------- phase 1: projections (weight-sharded) -------------
        xt = const_pool.tile([128, KCH * 2 * B], DT.bfloat16)
        nc.sync.dma_start(
            xt[:].rearrange("p (kc m) -> p kc m", kc=KCH),
            xT_d.ap().rearrange("(kc p) m -> p kc m", p=128),
        )
        b3t = const_pool.tile([B, 3 * SL], DT.float32)
        nc.sync.dma_start(b3t[:], b3_d.ap().partition_broadcast(B))

        # warm the exp activation table while projections run
        warm = const_pool.tile([1, 1], DT.float32)
        nc.scalar.activation(warm[:], b3t[0:1, 0:1], F.Exp)

        # all weights resident in SBUF (48 KB/partition)
        wqk = const_pool.tile([128, KCH * 1024], DT.bfloat16)
        wv = const_pool.tile([128, KCH * 512], DT.bfloat16)
        KG = 4
        for g in range(KCH // KG):
            nc.sync.dma_start(
                wqk[:, g * KG * 1024:(g + 1) * KG * 1024]
                    .rearrange("p (kc n) -> p kc n", kc=KG),
                wqk_d.ap()[g * KG * 128:(g + 1) * KG * 128, :]
                    .rearrange("(kc p) n -> p kc n", p=128))
        for g in range(2):
            nc.sync.dma_start(
                wv[:, g * 8 * 512:(g + 1) * 8 * 512]
                    .rearrange("p (kc n) -> p kc n", kc=8),
                wv_d.ap()[g * 8 * 128:(g + 1) * 8 * 128, :]
                    .rearrange("(kc p) n -> p kc n", p=128))

        qk_sb = const_pool.tile([B, 2 * SL], DT.float32)
        vp_sb = const_pool.tile([B, SL], DT.float32)
        with tc.tile_pool(name="psp", bufs=1, space="PSUM") as pp:
            ps0 = pp.tile([B, 2 * SL], DT.float32)
            ps1 = pp.tile([B, SL], DT.float32)
            # q/k first: they gate the attention phase
            for kc in range(KCH):
                xh = xt[:, kc * 2 * B:kc * 2 * B + B]
                xl = xt[:, kc * 2 * B + B:(kc + 1) * 2 * B]
                wh = wqk[:, kc * 1024:kc * 1024 + 512]
                wl = wqk[:, kc * 1024 + 512:(kc + 1) * 1024]
                for t, (lhsT, rhs) in enumerate([(xh, wh), (xh, wl), (xl, wh)]):
                    nc.tensor.matmul(ps0[:], lhsT, rhs,
                                     start=(kc == 0 and t == 0),
                                     stop=(kc == KCH - 1 and t == 2))
            nc.vector.tensor_add(qk_sb[:], ps0[:], b3t[:, 0:2 * SL])
            nc.sync.dma_start(cc1_in.ap(), qk_sb[:])
            nc.gpsimd.collective_compute(
                "AllToAll", OP.bypass,
                replica_groups=[list(range(NCORES))],
                ins=[cc1_in.ap()], outs=[cc1_out.ap()])

            # v projection; its AllToAll overlaps with early attention
            for kc in range(KCH):
                xh = xt[:, kc * 2 * B:kc * 2 * B + B]
                xl = xt[:, kc * 2 * B + B:(kc + 1) * 2 * B]
                wh = wv[:, kc * 512:kc * 512 + 256]
                wl = wv[:, kc * 512 + 256:(kc + 1) * 512]
                for t, (lhsT, rhs) in enumerate([(xh, wh), (xh, wl), (xl, wh)]):
                    nc.tensor.matmul(ps1[:], lhsT, rhs,
                                     start=(kc == 0 and t == 0),
                                     stop=(kc == KCH - 1 and t == 2))
            nc.vector.tensor_add(vp_sb[:], ps1[:], b3t[:, 2 * SL:3 * SL])
            nc.sync.dma_start(cc2_in.ap(), vp_sb[:])
            nc.gpsimd.collective_compute(
                "AllToAll", OP.bypass,
                replica_groups=[list(range(NCORES))],
                ins=[cc2_in.ap()], outs=[cc2_out.ap()])

        # core c now holds, for each d:
        #   cc1_out[4d + i] = (q | k)[batch 4c + i, 256d:256d+256]
        #   cc2_out[4d + i] = v'[batch 4c + i, 256d:256d+256]
        cc1 = cc1_out.ap()
        cc2 = cc2_out.ap()

        # ---------------- phase 2: attention (batch-parallel) ---------------
        apool = ctx.enter_context(tc.tile_pool(name="attn", bufs=3))
        epool = ctx.enter_context(tc.tile_pool(name="etile", bufs=10))
        with tc.tile_pool(name="psa", bufs=2, space="PSUM") as spp:
            for i in range(BL):
                # q in cols 0:16, v' in 16:32 ; i-index mapping: i = p*16 + c
                qv = apool.tile([128, 32], DT.float32, tag="qv")
                kb = apool.tile([128, SEQ], DT.float32, tag="kb")
                nc.sync.dma_start(
                    kb[:],
                    cc1[:, SL:2 * SL].rearrange(
                        "(d i) o -> i d o", i=BL)[i:i + 1].partition_broadcast(128))
                nc.sync.dma_start(
                    qv[:, 0:16],
                    cc1[:, 0:SL].rearrange(
                        "(d i) (pp c) -> i d pp c", i=BL, c=16)[i:i + 1])
                nc.sync.dma_start(
                    qv[:, 16:32],
                    cc2.rearrange(
                        "(d i) (pp c) -> i d pp c", i=BL, c=16)[i:i + 1])

                zt = apool.tile([128, KCH], DT.float32, tag="zt")
                rz = apool.tile([128, KCH], DT.float32, tag="rz")
                wt = apool.tile([128, KCH], DT.float16, tag="wt")
                psum = spp.tile([1, SEQ], DT.float32)
                for c in range(KCH):
                    e16 = epool.tile([128, SEQ], DT.float16)
                    nc.scalar.activation(
                        e16[:], kb[:], F.Exp,
                        scale=qv[:, c:c + 1], accum_out=zt[:, c:c + 1])
                    nc.vector.reciprocal(rz[:, c:c + 1], zt[:, c:c + 1])
                    nc.vector.tensor_mul(
                        wt[:, c:c + 1], qv[:, 16 + c:17 + c], rz[:, c:c + 1])
                    for js in range(4):
                        nc.tensor.matmul(
                            psum[0:1, js * 512:(js + 1) * 512],
                            wt[:, c:c + 1],
                            e16[:, js * 512:(js + 1) * 512],
                            start=(c == 0), stop=(c == KCH - 1),
                        )
                # epilogue: out_row = psum/VS + x_row  (all at partition 0)
                xrow = apool.tile([1, SEQ], DT.float32, tag="xrow")
                nc.sync.dma_start(xrow[:], xloc_d.ap()[i:i + 1, :])
                sarow = apool.tile([1, SEQ], DT.float32, tag="sarow")
                nc.vector.tensor_scalar(
                    sarow[:], psum[0:1, :], 1.0 / VS, None, op0=OP.mult)
                orow = apool.tile([1, SEQ], DT.float32, tag="orow")
                nc.vector.tensor_add(orow[:], sarow[:], xrow[:])
                nc.sync.dma_start(out_d.ap()[i:i + 1, :], orow[:])
    nc.compile()
    return nc


def _hilo(a):
    """Split fp32 array into (hi, lo) bf16 with a == hi + lo to ~2^-17."""
    import ml_dtypes
    hi = a.astype(ml_dtypes.bfloat16)
    lo = (a - hi.astype(np.float32)).astype(ml_dtypes.bfloat16)
    return hi, lo


def _prep_inputs(x, Wq, bq, Wk, bk, Wv, bv):
    x = np.ascontiguousarray(x, dtype=np.float32)
    xh, xl = _hilo(x.T)
    xT = np.concatenate([xh, xl], axis=1)  # [2048, 64] bf16 (hi | lo)
    in_maps = []
    for c in range(NCORES):
        sl = slice(SL * c, SL * (c + 1))
        wqk = np.concatenate([Wq[sl].T, Wk[sl].T], axis=1)  # [2048, 512] f32
        qh, ql = _hilo(np.ascontiguousarray(wqk, dtype=np.float32))
        wv = np.ascontiguousarray((Wv[sl] * VS).T, dtype=np.float32)
        vh, vl = _hilo(wv)
        b3 = np.concatenate([bq[sl], bk[sl], bv[sl] * VS])[None, :]
        in_maps.append({
            "xT": np.ascontiguousarray(xT),
            "wqk": np.ascontiguousarray(np.concatenate([qh, ql], axis=1)),
            "wv": np.ascontiguousarray(np.concatenate([vh, vl], axis=1)),
            "b3": np.ascontiguousarray(b3, dtype=np.float32),
            "xloc": np.ascontiguousarray(x[BL * c:BL * (c + 1)]),
        })
    return in_maps


def run_on_device(x, Wq, bq, Wk, bk, Wv, bv, **spmd_kwargs):
    if "nc" not in _CACHE:
        _CACHE["nc"] = _build()
    nc = _CACHE["nc"]
    in_maps = _prep_inputs(x, Wq, bq, Wk, bk, Wv, bv)
    res = run_bass_kernel_spmd(nc, in_maps, core_ids=list(range(NCORES)),
                               **spmd_kwargs)
    out = np.concatenate([res.results[c]["out"] for c in range(NCORES)], axis=0)
    return np.ascontiguousarray(out, dtype=np.float32), res


def kernel(x, Wq, bq, Wk, bk, Wv, bv):
    out, _ = run_on_device(x, Wq, bq, Wk, bk, Wv, bv)
    return out
